# revision 14
# baseline (speedup 1.0000x reference)
"""Self-contained Trainium2 Bass kernel for nn_CPINet_36850819400255.

Strategy: pure data parallelism over batch B=256 -> 8 cores x 32 samples.
Per core the dominant cost is the 3-layer 23x23 conv over [2048, 64] maps.
It is computed as 12 accumulating K=128 matmuls per 512-row block by
packing (kh-pair, d_in) into the contraction dim against a transposed,
zero-padded image whose partition rows 64..127 hold a copy shifted by one
column (so each matmul covers two kernel rows).  Two samples run
concurrently in PE column groups 0-63 / 64-127 (tile_position col tiling),
filling the whole 128x128 array.
"""

import sys

sys.path.insert(0, "/opt/trn_rl_repo")

import numpy as np

import concourse.bass as bass
import concourse.mybir as mybir
import concourse.tile as tile
from concourse import bacc
from concourse.bass_utils import run_bass_kernel_spmd
from concourse.masks import make_identity

F32 = mybir.dt.float32
I32 = mybir.dt.int32
AF = mybir.ActivationFunctionType
OP = mybir.AluOpType

NCORES = 8
B_TOT = 256
NS = B_TOT // NCORES          # samples per core
N = 128                       # atoms
L = 2048                      # amino length
D = 64
PAD = 11
XW = 2080                     # padded width of transposed conv image
EPS = 1e-6


def build_nc(nsamp=NS, stage=99, sub=99):
    """Build the single-core Bass program (SPMD across 8 cores).
    stage: 1=prep only, 2=+conv, 3=+attention, 99=full."""
    nc = bacc.Bacc("TRN2", target_bir_lowering=False, debug=True)

    # ---- DRAM I/O ----
    atoms_d = nc.dram_tensor("atoms", [nsamp, N], I32, kind="ExternalInput")
    amino_d = nc.dram_tensor("amino", [nsamp, L], I32, kind="ExternalInput")
    amask_d = nc.dram_tensor("amask", [nsamp, N], F32, kind="ExternalInput")
    pmask_d = nc.dram_tensor("pmask", [nsamp, L], F32, kind="ExternalInput")
    adj_d = nc.dram_tensor("adj", [nsamp, N, N], F32, kind="ExternalInput")
    embf_d = nc.dram_tensor("embf", [2000, D], F32, kind="ExternalInput")
    embw_d = nc.dram_tensor("embw", [10000, D], F32, kind="ExternalInput")
    wg_d = nc.dram_tensor("wg", [3, D + 1, D], F32, kind="ExternalInput")
    tk_d = nc.dram_tensor("tk", [3, 12, 128, D], F32, kind="ExternalInput")
    cb_d = nc.dram_tensor("cb", [3, 128], F32, kind="ExternalInput")
    wa_d = nc.dram_tensor("wa", [D + 1, D], F32, kind="ExternalInput")
    wo_d = nc.dram_tensor("wo", [2, 128, 128], F32, kind="ExternalInput")
    bo_d = nc.dram_tensor("bo", [2, 128], F32, kind="ExternalInput")
    wi_d = nc.dram_tensor("wi", [128, 2], F32, kind="ExternalInput")
    bi_d = nc.dram_tensor("bi", [2], F32, kind="ExternalInput")
    out_d = nc.dram_tensor("out", [2, nsamp], F32, kind="ExternalOutput")

    with tile.TileContext(nc) as tc:
        with (
            tc.tile_pool(name="cp", bufs=1) as cp,          # constants
            tc.tile_pool(name="xp", bufs=6) as xp,          # conv images
            tc.tile_pool(name="pp", bufs=3) as pp,          # psT / hsT
            tc.tile_pool(name="gp", bufs=2) as gp,          # gather staging
            tc.tile_pool(name="sm", bufs=3) as sm,          # small sbuf
            tc.tile_pool(name="pc", bufs=3, space="PSUM") as pc,   # conv psum
            tc.tile_pool(name="pa", bufs=3, space="PSUM") as pa,   # attn psum
            tc.tile_pool(name="pz", bufs=2, space="PSUM") as pz,   # small psum
        ):
            # ---------- constants ----------
            ident = cp.tile([128, 128], F32, tag="ident")
            make_identity(nc, ident[:])
            ones_c = cp.tile([128, D], F32, tag="ones_c")
            nc.vector.memset(ones_c[:], 1.0)
            ones_r = cp.tile([1, D], F32, tag="ones_r")
            nc.vector.memset(ones_r[:], 1.0)

            tk_sb = cp.tile([128, 3 * 12 * D], F32, tag="tk")
            for i in range(3):
                for j in range(12):
                    k = i * 12 + j
                    nc.sync.dma_start(tk_sb[:, k * D:(k + 1) * D], tk_d[i, j])
            wg_sb = cp.tile([D + 1, 3 * D], F32, tag="wg")
            for i in range(3):
                nc.sync.dma_start(wg_sb[:, i * D:(i + 1) * D], wg_d[i])
            wa_sb = cp.tile([D + 1, D], F32, tag="wa")
            nc.sync.dma_start(wa_sb[:], wa_d[:])
            cb_sb = cp.tile([128, 3], F32, tag="cb")
            for i in range(3):
                nc.sync.dma_start(cb_sb[:, i:i + 1], cb_d[i, :, None])
            wo_sb = cp.tile([128, 256], F32, tag="wo")
            for j in range(2):
                nc.sync.dma_start(wo_sb[:, j * 128:(j + 1) * 128], wo_d[j])
            bo_sb = cp.tile([128, 2], F32, tag="bo")
            for j in range(2):
                nc.sync.dma_start(bo_sb[:, j:j + 1], bo_d[j, :, None])
            wi_sb = cp.tile([128, 2], F32, tag="wi")
            nc.sync.dma_start(wi_sb[:], wi_d[:])
            bi_sb = cp.tile([2, 1], F32, tag="bi")
            nc.sync.dma_start(bi_sb[:], bi_d[:, None])

            catC = cp.tile([128, nsamp], F32, tag="cat")

            def prep_sample(s):
                """Gathers + GNN + compound + conv-layer-0 image for sample s.
                Returns (X, cT_ext, prc, pm_row)."""
                # ---- atom side ----
                aidx = sm.tile([N, 1], I32, tag="aidx")
                nc.sync.dma_start(aidx[:], atoms_d[s, :, None])
                am_col = sm.tile([N, 1], F32, tag="amcol")
                nc.sync.dma_start(am_col[:], amask_d[s, :, None])
                xsF = sm.tile([N, 128], F32, tag="xs")
                nc.vector.memset(xsF[:, D:], 1.0)
                nc.gpsimd.indirect_dma_start(
                    out=xsF[:, 0:D], out_offset=None, in_=embf_d[:],
                    in_offset=bass.IndirectOffsetOnAxis(ap=aidx[:, :1], axis=0),
                )
                adjt = sm.tile([N, N], F32, tag="adj")
                nc.sync.dma_start(adjt[:], adj_d[s])
                pA = pz.tile([128, 512], F32, tag="ss")
                nc.tensor.transpose(pA[:, 0:N], adjt[:], ident[:])
                AT = sm.tile([N, N], F32, tag="AT")
                nc.vector.tensor_copy(AT[:], pA[:, 0:N])

                # ---- GNN ----
                for i in range(3):
                    xsT = sm.tile([D + 1, N], F32, tag="xst")
                    nc.vector.memset(xsT[D:D + 1, :], 1.0)
                    pT = pz.tile([128, 512], F32, tag="ss")
                    nc.tensor.transpose(pT[0:D, 0:N], xsF[:, 0:D], ident[:])
                    nc.scalar.copy(xsT[0:D, :], pT[0:D, 0:N])
                    ph = pz.tile([128, 512], F32, tag="ss")
                    nc.tensor.matmul(ph[0:N, 0:D], xsT[:], wg_sb[:, i * D:(i + 1) * D],
                                     start=True, stop=True)
                    hs = sm.tile([N, D], F32, tag="hs")
                    nc.scalar.activation(hs[:], ph[0:N, 0:D], AF.Relu)
                    px = pz.tile([128, 512], F32, tag="ss")
                    nc.tensor.matmul(px[0:N, 0:D], AT[:], hs[:], start=True, stop=True)
                    xsF2 = sm.tile([N, 128], F32, tag="xs")
                    nc.vector.memset(xsF2[:, D:], 1.0)
                    nc.vector.tensor_add(xsF2[:, 0:D], px[0:N, 0:D], xsF[:, 0:D])
                    xsF = xsF2

                # ---- compound (masked mean over atoms) ----
                pcm = pz.tile([128, 512], F32, tag="ss")
                nc.tensor.matmul(pcm[0:128, 0:1], xsF[:], am_col[:], start=True, stop=True)
                dn = sm.tile([D, 1], F32, tag="dn")
                nc.vector.tensor_scalar_add(dn[:], pcm[D:128, 0:1], EPS)
                rc = sm.tile([D, 1], F32, tag="rc")
                nc.vector.reciprocal(rc[:], dn[:])
                cT = sm.tile([D + 1, 1], F32, tag="ct")
                nc.vector.memset(cT[D:D + 1, :], 1.0)
                nc.vector.tensor_tensor(cT[0:D, :], pcm[0:D, 0:1], rc[:], op=OP.mult)
                nc.vector.tensor_copy(catC[0:D, s:s + 1], cT[0:D, :])

                # ---- protein mask ----
                pm16 = sm.tile([128, 16], F32, tag="pm16")
                nc.sync.dma_start(pm16[:], pmask_d[s].rearrange("(p t) -> p t", t=16))
                pmj = sm.tile([128, 16], F32, tag="pmj")
                pmsum = sm.tile([128, 1], F32, tag="pmsum")
                nc.scalar.activation(pmj[:], pm16[:], AF.Copy, accum_out=pmsum[:])
                ppd = pz.tile([128, 512], F32, tag="ss")
                nc.tensor.matmul(ppd[0:D, 0:1], ones_c[:], pmsum[:], start=True, stop=True)
                pdn = sm.tile([D, 1], F32, tag="pdn")
                nc.vector.tensor_scalar_add(pdn[:], ppd[0:D, 0:1], EPS)
                prc = sm.tile([D, 1], F32, tag="prc")
                nc.vector.reciprocal(prc[:], pdn[:])
                pm_row = sm.tile([1, L], F32, tag="pmrow")
                nc.sync.dma_start(pm_row[:], pmask_d[s, None, :])

                # ---- word gather + conv layer-0 image ----
                midx = sm.tile([128, 16], I32, tag="midx")
                nc.sync.dma_start(midx[:], amino_d[s].rearrange("(p t) -> p t", t=16))
                gt = gp.tile([128, 16 * D], F32, tag="gt")
                for t in range(16):
                    nc.gpsimd.indirect_dma_start(
                        out=gt[:, t * D:(t + 1) * D], out_offset=None, in_=embw_d[:],
                        in_offset=bass.IndirectOffsetOnAxis(ap=midx[:, t:t + 1], axis=0),
                    )
                X = xp.tile([128, XW], F32, tag="X")
                nc.vector.memset(X[0:D, 0:PAD], 0.0)
                nc.vector.memset(X[0:D, PAD + L:XW], 0.0)
                nc.vector.memset(X[D:128, 0:PAD - 1], 0.0)
                nc.vector.memset(X[D:128, PAD - 1 + L:XW], 0.0)
                for t in range(16):
                    pg = pz.tile([128, 512], F32, tag="ss")
                    nc.tensor.transpose(pg[0:D, 0:128], gt[:, t * D:(t + 1) * D], ident[:])
                    # gathered row p of tile t is amino position l = p*16 + t
                    dst0 = X[0:D, PAD + t: PAD + t + 16 * 128].rearrange(
                        "p (l u) -> p l u", u=16)[:, :, 0]
                    dst1 = X[D:128, PAD - 1 + t: PAD - 1 + t + 16 * 128].rearrange(
                        "p (l u) -> p l u", u=16)[:, :, 0]
                    nc.scalar.copy(dst0, pg[0:D, 0:128])
                    nc.vector.tensor_copy(dst1, pg[0:D, 0:128])
                return X, cT, prc, pm_row

            def conv_pair(XA, XB):
                """3 conv layers on a sample pair; returns (psTA, psTB)."""
                for i in range(3):
                    last = i == 2
                    if last:
                        oA = pp.tile([D + 1, L], F32, tag="psT")
                        oB = pp.tile([D + 1, L], F32, tag="psT")
                        nc.vector.memset(oA[D:D + 1, :], 1.0)
                        nc.vector.memset(oB[D:D + 1, :], 1.0)
                    else:
                        oA = xp.tile([128, XW], F32, tag="X")
                        oB = xp.tile([128, XW], F32, tag="X")
                        for o in (oA, oB):
                            nc.vector.memset(o[0:D, 0:PAD], 0.0)
                            nc.vector.memset(o[0:D, PAD + L:XW], 0.0)
                            nc.vector.memset(o[D:128, 0:PAD - 1], 0.0)
                            nc.vector.memset(o[D:128, PAD - 1 + L:XW], 0.0)
                    for b in range(4):
                        pv = pc.tile([128, 512], F32, tag="cv")
                        for j in range(12):
                            w = tk_sb[:, (i * 12 + j) * D:(i * 12 + j + 1) * D]
                            st, sp = j == 0, j == 11
                            c0 = 2 * j + b * 512
                            nc.tensor.matmul(pv[0:D, :], w, XA[:, c0:c0 + 512],
                                             start=st, stop=sp, skip_group_check=True)
                            nc.tensor.matmul(pv[D:128, :], w, XB[:, c0:c0 + 512],
                                             start=st, stop=sp, skip_group_check=True)
                        bl = b * 512
                        if last:
                            nc.scalar.activation(oA[0:D, bl:bl + 512], pv[0:D, :],
                                                 AF.Relu, bias=cb_sb[0:D, i:i + 1])
                            nc.vector.tensor_scalar(
                                oB[0:D, bl:bl + 512], pv[D:128, :],
                                cb_sb[D:128, i:i + 1], 0.0, op0=OP.add, op1=OP.max)
                        else:
                            nc.scalar.activation(
                                oA[0:D, PAD + bl: PAD + bl + 512], pv[0:D, :],
                                AF.Relu, bias=cb_sb[0:D, i:i + 1])
                            nc.vector.tensor_scalar(
                                oA[D:128, PAD - 1 + bl: PAD - 1 + bl + 512], pv[0:D, :],
                                cb_sb[0:D, i:i + 1], 0.0, op0=OP.add, op1=OP.max)
                            nc.scalar.activation(
                                oB[0:D, PAD + bl: PAD + bl + 512], pv[D:128, :],
                                AF.Relu, bias=cb_sb[D:128, i:i + 1])
                            nc.vector.tensor_scalar(
                                oB[D:128, PAD - 1 + bl: PAD - 1 + bl + 512], pv[D:128, :],
                                cb_sb[D:128, i:i + 1], 0.0, op0=OP.add, op1=OP.max)
                    XA, XB = oA, oB
                return XA, XB

            def attention(s, psT, cT, prc, pm_row, sub=99):
                hsT = pp.tile([D, L], F32, tag="hsT")
                for b in range(4):
                    ph = pa.tile([128, 512], F32, tag="at")
                    nc.tensor.matmul(ph[0:D, :], wa_sb[:], psT[:, b * 512:(b + 1) * 512],
                                     start=True, stop=True)
                    nc.scalar.activation(hsT[:, b * 512:(b + 1) * 512], ph[0:D, :], AF.Relu)
                if sub < 2:
                    return
                pq = pz.tile([128, 512], F32, tag="ss")
                nc.tensor.matmul(pq[0:D, 0:1], wa_sb[:], cT[:], start=True, stop=True)
                hq = sm.tile([D, 1], F32, tag="hq")
                nc.scalar.activation(hq[:], pq[0:D, 0:1], AF.Relu)
                if sub < 3:
                    return
                w_row = sm.tile([1, L], F32, tag="wrow")
                for b in range(4):
                    pw = pa.tile([128, 512], F32, tag="at")
                    nc.tensor.matmul(pw[0:1, :], hq[:],
                                     hsT[:, b * 512:(b + 1) * 512], start=True, stop=True)
                    nc.scalar.activation(w_row[:, b * 512:(b + 1) * 512],
                                         pw[0:1, :], AF.Tanh)
                if sub < 4:
                    return
                nc.vector.tensor_tensor(w_row[:], w_row[:], pm_row[:], op=OP.mult)
                if sub < 5:
                    return
                pacc = sm.tile([D, 4], F32, tag="pacc")
                for b in range(4):
                    pwb = pa.tile([128, 512], F32, tag="at")
                    nc.tensor.matmul(pwb[0:D, :], ones_r[:],
                                     w_row[:, b * 512:(b + 1) * 512], start=True, stop=True)
                    scr = sm.tile([D, 512], F32, tag="scr")
                    nc.vector.tensor_tensor(scr[:], hsT[:, b * 512:(b + 1) * 512],
                                            pwb[0:D, :], op=OP.mult)
                    sj = sm.tile([D, 512], F32, tag="sj")
                    nc.scalar.activation(sj[:], scr[:], AF.Copy,
                                         accum_out=pacc[:, b:b + 1])
                if sub < 6:
                    return
                pj = sm.tile([D, 4], F32, tag="pj")
                praw = sm.tile([D, 1], F32, tag="praw")
                nc.scalar.activation(pj[:], pacc[:], AF.Copy, accum_out=praw[:])
                nc.vector.tensor_tensor(catC[D:128, s:s + 1], praw[:], prc[:], op=OP.mult)

            # ================= main loop =================
            for t in range(nsamp // 2):
                s0, s1 = 2 * t, 2 * t + 1
                XA, cT0, prc0, pmr0 = prep_sample(s0)
                XB, cT1, prc1, pmr1 = prep_sample(s1)
                if stage < 2:
                    continue
                psTA, psTB = conv_pair(XA, XB)
                if stage < 3:
                    if t == 0:
                        nc.sync.dma_start(out_d[:], psTA[0:2, 0:nsamp])
                    continue
                attention(s0, psTA, cT0, prc0, pmr0, sub)
                attention(s1, psTB, cT1, prc1, pmr1, sub)

            if stage < 2 or (stage >= 3 and sub < 99):
                nc.sync.dma_start(out_d[:], catC[0:2, 0:nsamp])
            if stage >= 3 and sub >= 99:
                # ================= output MLP =================
                p1 = pz.tile([128, 512], F32, tag="ss")
                nc.tensor.matmul(p1[0:128, 0:nsamp], wo_sb[:, 0:128], catC[:],
                                 start=True, stop=True)
                cat1 = sm.tile([128, nsamp], F32, tag="cat1")
                nc.scalar.activation(cat1[:], p1[0:128, 0:nsamp], AF.Relu,
                                     bias=bo_sb[:, 0:1])
                p2 = pz.tile([128, 512], F32, tag="ss")
                nc.tensor.matmul(p2[0:128, 0:nsamp], wo_sb[:, 128:256], cat1[:],
                                 start=True, stop=True)
                cat2 = sm.tile([128, nsamp], F32, tag="cat2")
                nc.scalar.activation(cat2[:], p2[0:128, 0:nsamp], AF.Relu,
                                     bias=bo_sb[:, 1:2])
                p3 = pz.tile([128, 512], F32, tag="ss")
                nc.tensor.matmul(p3[0:2, 0:nsamp], wi_sb[:], cat2[:],
                                 start=True, stop=True)
                outS = sm.tile([2, nsamp], F32, tag="os")
                nc.scalar.activation(outS[:], p3[0:2, 0:nsamp], AF.Identity,
                                     bias=bi_sb[:])
                nc.sync.dma_start(out_d[:], outS[:])

    nc.compile()
    return nc


def build_tk(conv_k):
    """conv_k [3, 23, 23] -> TK [3, 12, 128, 64] banded matrices.
    TK[i][j][(s, d_in), d_out] = conv_k[i, 2j+s, d_in - d_out + 11]."""
    TK = np.zeros((3, 12, 128, D), np.float32)
    ck = np.asarray(conv_k, np.float32)
    for i in range(3):
        for kh in range(23):
            j, sl = divmod(kh, 2)
            for do in range(D):
                lo = max(0, do - PAD)
                hi = min(D, do + PAD + 1)
                TK[i, j, sl * D + lo: sl * D + hi, do] = \
                    ck[i, kh, lo - do + PAD: hi - do + PAD]
    return TK


def make_in_maps(inputs, nsamp=NS, ncores=NCORES):
    f32 = lambda x: np.ascontiguousarray(np.asarray(x), dtype=np.float32)
    i32 = lambda x: np.ascontiguousarray(np.asarray(x), dtype=np.int32)

    wg = np.concatenate(
        [np.transpose(f32(inputs["W_gnn"]), (0, 2, 1)),
         f32(inputs["b_gnn"])[:, None, :]], axis=1)            # [3, 65, 64]
    tk = build_tk(inputs["conv_k"])
    cb = np.repeat(f32(inputs["conv_b"])[:, None], 128, axis=1)  # [3, 128]
    wa = np.concatenate([f32(inputs["W_att"]).T,
                         f32(inputs["b_att"])[None, :]], axis=0)  # [65, 64]
    wo = np.ascontiguousarray(np.transpose(f32(inputs["W_out"]), (0, 2, 1)))
    wi = np.ascontiguousarray(f32(inputs["W_int"]).T)            # [128, 2]

    shared = dict(
        embf=f32(inputs["emb_fp"]), embw=f32(inputs["emb_word"]),
        wg=wg, tk=tk, cb=cb, wa=wa, wo=wo,
        bo=f32(inputs["b_out"]), wi=wi, bi=f32(inputs["b_int"]),
    )
    atoms = i32(inputs["atoms"])
    amino = i32(inputs["amino"])
    amask = f32(inputs["atoms_mask"])
    pmask = f32(inputs["amino_mask"])
    adj = f32(inputs["adjacency"])

    in_maps = []
    for c in range(ncores):
        sl = slice(c * nsamp, (c + 1) * nsamp)
        m = dict(shared)
        m.update(atoms=atoms[sl], amino=amino[sl], amask=amask[sl],
                 pmask=pmask[sl], adj=adj[sl])
        in_maps.append(m)
    return in_maps


_NC_CACHE = {}


def _get_nc(nsamp=NS):
    if nsamp not in _NC_CACHE:
        _NC_CACHE[nsamp] = build_nc(nsamp)
    return _NC_CACHE[nsamp]


def kernel(**inputs):
    nc = _get_nc(NS)
    in_maps = make_in_maps(inputs, NS, NCORES)
    res = run_bass_kernel_spmd(nc, in_maps, core_ids=list(range(NCORES)))
    out = np.concatenate([np.asarray(r["out"]).T for r in res.results], axis=0)
    return np.ascontiguousarray(out, dtype=np.float32)
